# revision 17
# baseline (speedup 1.0000x reference)
"""Multi-head attention block (QKV proj -> softmax attention -> out proj) for
Trainium2, SPMD across 8 NeuronCores.

Sharding: batch (B=2) x head-groups (4 groups of 4 heads). Core c handles
batch c//4 and heads [4*(c%4), 4*(c%4)+4). Each core computes its partial
output contribution (context @ wo_slice.T); the host sums the 4 head-group
partials per batch (tensor-parallel row-sharded wo => the all-reduce is the
host-side gather).

All matmuls run in bf16 with fp32 PSUM accumulation. Layout choices keep the
tensor engine free of bookkeeping work:

  - V is projected directly into [key, d] orientation (x tile stationary,
    wvT moving), so no transposes are needed before the PV matmul.
  - Scores are computed transposed ([key, q]) so softmax exp output feeds
    the PV matmul directly as the moving operand.
  - Softmax denominators are per-query sums over keys (= partitions). They
    are accumulated over key tiles on the vector engine, transposed through
    the DMA x-bar, and finished with a free-axis reduce - zero PE cycles.
  - Context normalization happens on [q, d]-transposed blocks with a
    per-partition reciprocal scalar, then transposes back for the output
    projection (context is tiny vs. probs, so this is cheap).
  - The output projection runs as dense PE-only phases (one per 1024-token
    half) with the stationary context tile reused across 4 matmuls.

Per-core kernel DRAM I/O (everything [partition=128, free]):
  xT   [2048, 2048] bf16   x[b].T             (feature k on partitions)
  wqT/wkT/wvT [2048, 512]  w[heads_slice].T   (k on partitions)
  woT  [512, 2048]  bf16   wo[:, slice].T     (local d on partitions)
  out  [2048, 2048] fp32   partial output for batch b
"""

import sys

if "/opt/trn_rl_repo" not in sys.path:
    sys.path.insert(0, "/opt/trn_rl_repo")

from contextlib import ExitStack

import ml_dtypes
import numpy as np

import concourse.bacc as bacc
import concourse.tile as tile
from concourse import mybir
from concourse.bass_utils import run_bass_kernel_spmd

BF16 = mybir.dt.bfloat16
F32 = mybir.dt.float32

B, S, DIM = 2, 2048, 2048
HEADS, HD = 16, 128
P = 128
N_CORES = 8
HGROUPS = 4  # head groups (second shard axis is batch)
HPC = HEADS // HGROUPS  # heads per core = 4
DL = HPC * HD  # local head dims per core = 512
SCALE = 1.0 / float(np.sqrt(HD))

NK = DIM // P  # 16 contraction tiles for the projections
NMC = S // 512  # 4 token chunks in the projection phase
NT = S // P  # 16 key tiles
QW = 1024  # query block width in the attention phase
NJB = S // QW  # 2 query blocks
NJT = QW // P  # 8 query tiles per block
NE = DIM // 512  # 4 output-dim chunks

_PROGRAM_CACHE = {}


def _emit(nc, tc, xT, wqT, wkT, wvT, woT, maskf, out):
    with_mask = maskf is not None
    EXP = mybir.ActivationFunctionType.Exp
    AXX = mybir.AxisListType.X
    ADD = mybir.AluOpType.add

    with ExitStack() as octx:
        planes = octx.enter_context(tc.tile_pool(name="planes", bufs=1))
        q_sb = [planes.tile([P, S], BF16, tag=f"q{h}", name=f"q{h}") for h in range(HPC)]
        k_sb = [planes.tile([P, S], BF16, tag=f"k{h}", name=f"k{h}") for h in range(HPC)]
        cx_sb = [planes.tile([P, S], BF16, tag=f"cx{h}", name=f"cx{h}") for h in range(HPC)]
        vv_sb = planes.tile([P, NT, DL], BF16, tag="vv", name="vv")
        wo_sb = planes.tile([P, HPC, DIM], BF16, tag="wo", name="wo")

        dmae = [nc.sync, nc.scalar, nc.gpsimd]

        # ---------------- Phase 1: QKV projections ----------------
        with ExitStack() as ctx:
            wpool = ctx.enter_context(tc.tile_pool(name="wqkv", bufs=1))
            w_sb = {
                name: wpool.tile([P, NK * DL], BF16, tag=f"w{name}", name=f"w{name}")
                for name in ("q", "k", "v")
            }
            xpool = ctx.enter_context(tc.tile_pool(name="xt", bufs=2 * NK))
            pq = ctx.enter_context(tc.tile_pool(name="ps_qkv", bufs=2, space="PSUM"))

            qi = 0
            for mc in range(NMC):
                xts = []
                for kt in range(NK):
                    t = xpool.tile([P, 512], BF16, tag="xt")
                    # kt-interleaved issue order across all three queues so
                    # delivery tracks the kt-major consumption order below
                    dmae[qi % 3].dma_start(
                        t[:], xT[kt * P : (kt + 1) * P, mc * 512 : (mc + 1) * 512]
                    )
                    qi += 1
                    xts.append(t)
                    if mc == 0:
                        dmae[qi % 3].dma_start(
                            w_sb["q"][:, kt * DL : (kt + 1) * DL],
                            wqT[kt * P : (kt + 1) * P, :],
                        )
                        qi += 1
                if mc == 0:
                    for name, srct in (("k", wkT), ("v", wvT)):
                        for kt in range(NK):
                            dmae[qi % 3].dma_start(
                                w_sb[name][:, kt * DL : (kt + 1) * DL],
                                srct[kt * P : (kt + 1) * P, :],
                            )
                            qi += 1
                # kt-major accumulation: 4 live PSUM banks per projection, so
                # each x tile is consumed right as it lands (no burst demand)
                for name, plane_list in (("q", q_sb), ("k", k_sb)):
                    pss = [
                        pq.tile([P, 512], F32, tag=f"ps{i}", name=f"ps{i}")
                        for i in range(4)
                    ]
                    for kt in range(NK):
                        for h in range(HPC):
                            nc.tensor.matmul(
                                pss[h][:],
                                w_sb[name][:, kt * DL + h * P : kt * DL + (h + 1) * P],
                                xts[kt][:],
                                start=(kt == 0),
                                stop=(kt == NK - 1),
                            )
                    for h in range(HPC):
                        nc.any.tensor_copy(
                            plane_list[h][:, mc * 512 : (mc + 1) * 512], pss[h][:]
                        )
                # v in [token(=key), d] orientation: x tile stationary
                pss = [
                    pq.tile([P, 512], F32, tag=f"ps{i}", name=f"ps{i}")
                    for i in range(4)
                ]
                for kt in range(NK):
                    for tt in range(4):
                        nc.tensor.matmul(
                            pss[tt][:],
                            xts[kt][:, tt * P : (tt + 1) * P],
                            w_sb["v"][:, kt * DL : (kt + 1) * DL],
                            start=(kt == 0),
                            stop=(kt == NK - 1),
                        )
                for tt in range(4):
                    nc.any.tensor_copy(vv_sb[:, mc * 4 + tt, :], pss[tt][:])

        # ------- Phase 2: attention (scoresT form), per query-block -------
        # scoresT(nt) = kT(nt)^T @ qT -> exp -> probsT [key, q] feeds PV
        # directly. Denominators: DVE accumulates probsT over key tiles,
        # x-bar transposes the sums, free-axis reduce + reciprocal gives
        # per-query scalars in column form. Context is normalized on [q, d]
        # blocks and transposed back. Out-projection runs as its own dense
        # PE phase after each query block.
        for jb in range(NJB):
            q0 = jb * QW
            with ExitStack() as ctx:
                ss_pool = ctx.enter_context(
                    tc.tile_pool(name="ss", bufs=3, space="PSUM")
                )
                cxp_pool = ctx.enter_context(
                    tc.tile_pool(name="cxp", bufs=1, space="PSUM")
                )
                pbt_pool = ctx.enter_context(tc.tile_pool(name="pbt", bufs=6))
                acc_pool = ctx.enter_context(tc.tile_pool(name="acc", bufs=2))
                tr_pool = ctx.enter_context(tc.tile_pool(name="tr", bufs=4))
                st_pool = ctx.enter_context(tc.tile_pool(name="st", bufs=4))
                if with_mask:
                    mpool = ctx.enter_context(tc.tile_pool(name="mask", bufs=3))

                if jb == 0:
                    # wo is first needed by the out-projection ~100us later;
                    # load it here so it never delays the phase-1 x/w DMAs
                    for h in range(HPC):
                        nc.gpsimd.dma_start(
                            wo_sb[:, h, :], woT[h * P : (h + 1) * P, :]
                        )

                for h in range(HPC):
                    # two bf16 accumulator chains for the softmax denominators
                    acc = [
                        acc_pool.tile([P, QW], BF16, tag=f"acc{i}", name=f"acc{i}")
                        for i in range(2)
                    ]
                    cxp = cxp_pool.tile([P, 2, 512], F32, tag="cxp", name="cxp")
                    for nt in range(NT):
                        ss = ss_pool.tile([P, QW], F32, tag="ss", name="ss")
                        kst = k_sb[h][:, nt * P : (nt + 1) * P]
                        nc.tensor.matmul(
                            ss[:, 0:512], kst, q_sb[h][:, q0 : q0 + 512],
                            start=True, stop=True,
                        )
                        nc.tensor.matmul(
                            ss[:, 512:1024], kst, q_sb[h][:, q0 + 512 : q0 + QW],
                            start=True, stop=True,
                        )
                        if with_mask:
                            mt = mpool.tile([P, QW], F32, tag="mt", name="mt")
                            nc.gpsimd.dma_start(
                                mt[:], maskf[nt * P : (nt + 1) * P, q0 : q0 + QW]
                            )
                            nc.vector.tensor_add(ss[:], ss[:], mt[:])
                        pbt = pbt_pool.tile([P, QW], BF16, tag="pbt", name="pbt")
                        nc.scalar.activation(pbt[:], ss[:], EXP, scale=SCALE)
                        a = acc[nt % 2]
                        if nt < 2:
                            nc.vector.tensor_copy(a[:], pbt[:])
                        else:
                            nc.vector.tensor_add(a[:], a[:], pbt[:])
                        vst = vv_sb[:, nt, h * P : (h + 1) * P]
                        nc.tensor.matmul(
                            cxp[:, 0, :], vst, pbt[:, 0:512],
                            start=(nt == 0), stop=(nt == NT - 1),
                        )
                        nc.tensor.matmul(
                            cxp[:, 1, :], vst, pbt[:, 512:1024],
                            start=(nt == 0), stop=(nt == NT - 1),
                        )
                    # tail: denominators + context normalization, in
                    # 512-wide halves so the first tokens' context lands
                    # quickly (shrinks the stall before the out-projection)
                    # merge + normalize muls run on gpsimd: the vector engine
                    # is at capacity with the acc chains, and queue congestion
                    # here stalls the next unit's exp cadence
                    accb = acc_pool.tile([P, QW], BF16, tag="accb", name="accb")
                    nc.gpsimd.tensor_add(accb[:], acc[0][:], acc[1][:])
                    for hv in range(2):
                        c0 = hv * 512
                        accT = tr_pool.tile([P, 4, P], BF16, tag="accT", name="accT")
                        nc.sync.dma_start(
                            accT[:], accb[:, c0 : c0 + 512], transpose=True
                        )
                        ctmp = acc_pool.tile([P, 512], BF16, tag="ctmp", name="ctmp")
                        nc.vector.tensor_copy(ctmp[:], cxp[:, hv, :])
                        cn = tr_pool.tile([P, 4, P], BF16, tag="cn", name="cn")
                        nc.sync.dma_start(cn[:], ctmp[:], transpose=True)
                        den = st_pool.tile([P, 4], F32, tag="den", name="den")
                        nc.vector.tensor_reduce(den[:], accT[:], axis=AXX, op=ADD)
                        rec = st_pool.tile([P, 4], F32, tag="rec", name="rec")
                        nc.vector.reciprocal(rec[:], den[:])
                        cnn = tr_pool.tile([P, 4, P], BF16, tag="cnn", name="cnn")
                        for j in range(4):
                            nc.gpsimd.tensor_scalar_mul(
                                cnn[:, j, :], cn[:, j, :], rec[:, j : j + 1]
                            )
                        cx_dst = cx_sb[h][:, q0 + c0 : q0 + c0 + 512].rearrange(
                            "p (a b) -> p a b", a=4
                        )
                        nc.sync.dma_start(cx_dst, cnn[:], transpose=True)

            # ---- out projection for this query block (dense PE phase) ----
            with ExitStack() as ctx:
                po = ctx.enter_context(tc.tile_pool(name="po", bufs=2, space="PSUM"))
                obp = ctx.enter_context(tc.tile_pool(name="ob", bufs=6))
                for tt in range(NJT):
                    t0 = q0 + tt * P
                    pst = po.tile([P, NE, 512], F32, tag="pst", name="pst")
                    for h in range(HPC):
                        cst = cx_sb[h][:, t0 : t0 + P]
                        for ec in range(NE):
                            nc.tensor.matmul(
                                pst[:, ec, :],
                                cst,
                                wo_sb[:, h, ec * 512 : (ec + 1) * 512],
                                start=(h == 0),
                                stop=(h == HPC - 1),
                            )
                    for ec in range(NE):
                        # bf16 partial outputs (the host sums head-group
                        # partials in fp32): halves the output write traffic
                        ob = obp.tile([P, 512], BF16, tag="ob", name="ob")
                        # split PSUM->SBUF copies across vector+scalar so
                        # neither engine paces the PE here
                        if ec % 2:
                            nc.vector.tensor_copy(ob[:], pst[:, ec, :])
                        else:
                            nc.scalar.copy(ob[:], pst[:, ec, :])
                        dmae[(tt * NE + ec) % 3].dma_start(
                            out[t0 : t0 + P, ec * 512 : (ec + 1) * 512], ob[:]
                        )


def _build(with_mask: bool):
    nc = bacc.Bacc("TRN2")
    xT = nc.dram_tensor("xT", [DIM, S], BF16, kind="ExternalInput")
    wqT = nc.dram_tensor("wqT", [DIM, DL], BF16, kind="ExternalInput")
    wkT = nc.dram_tensor("wkT", [DIM, DL], BF16, kind="ExternalInput")
    wvT = nc.dram_tensor("wvT", [DIM, DL], BF16, kind="ExternalInput")
    woT = nc.dram_tensor("woT", [DL, DIM], BF16, kind="ExternalInput")
    maskf = (
        nc.dram_tensor("maskf", [S, S], F32, kind="ExternalInput")
        if with_mask
        else None
    )
    out = nc.dram_tensor("out", [S, DIM], BF16, kind="ExternalOutput")
    with tile.TileContext(nc) as tc:
        _emit(nc, tc, xT, wqT, wkT, wvT, woT, maskf, out)
    nc.finalize()
    return nc


def _get_program(with_mask: bool):
    if with_mask not in _PROGRAM_CACHE:
        _PROGRAM_CACHE[with_mask] = _build(with_mask)
    return _PROGRAM_CACHE[with_mask]


def _prep_in_maps(x, mask, wq, wk, wv, wo, with_mask):
    bf = ml_dtypes.bfloat16
    f32 = np.float32
    xTs = [np.ascontiguousarray(x[b].T.astype(bf)) for b in range(B)]
    if with_mask:
        maskf = np.ascontiguousarray(mask[0, 0].T.astype(f32) / SCALE)
    in_maps = []
    for c in range(N_CORES):
        b = c // HGROUPS
        g = c % HGROUPS
        sl = slice(g * DL, (g + 1) * DL)
        m = {
            "xT": xTs[b],
            "wqT": np.ascontiguousarray(wq[sl, :].T.astype(bf)),
            "wkT": np.ascontiguousarray(wk[sl, :].T.astype(bf)),
            "wvT": np.ascontiguousarray(wv[sl, :].T.astype(bf)),
            "woT": np.ascontiguousarray(wo[:, sl].T.astype(bf)),
        }
        if with_mask:
            m["maskf"] = maskf
        in_maps.append(m)
    return in_maps


def run_sharded(x, mask, wq, wk, wv, wo, trace=False, trace_kwargs=None):
    """Run the SPMD kernel; returns (full_output, BassKernelResults)."""
    with_mask = bool(np.any(np.asarray(mask)))
    nc = _get_program(with_mask)
    in_maps = _prep_in_maps(
        np.asarray(x), np.asarray(mask), np.asarray(wq), np.asarray(wk),
        np.asarray(wv), np.asarray(wo), with_mask,
    )
    kw = {}
    if trace:
        kw["trace"] = True
        if trace_kwargs:
            kw["trace_kwargs"] = trace_kwargs
    res = run_bass_kernel_spmd(nc, in_maps, list(range(N_CORES)), **kw)
    out = np.zeros((B, S, DIM), np.float32)
    for c in range(N_CORES):
        out[c // HGROUPS] += res.results[c]["out"].astype(np.float32)
    return out, res


def kernel(**inputs):
    out, _ = run_sharded(
        inputs["x"], inputs["mask"], inputs["wq"], inputs["wk"], inputs["wv"],
        inputs["wo"],
    )
    return out


# revision 21
# speedup vs baseline: 1.2157x; 1.2157x over previous
"""Multi-head attention block (QKV proj -> softmax attention -> out proj) for
Trainium2, SPMD across 8 NeuronCores.

Sharding: batch (B=2) x head-groups (4 groups of 4 heads). Core c handles
batch c//4 and heads [4*(c%4), 4*(c%4)+4). Each core computes its partial
output contribution (context @ wo_slice.T); the host sums the 4 head-group
partials per batch (tensor-parallel row-sharded wo => the all-reduce is the
host-side gather).

All matmuls run in bf16 with fp32 PSUM accumulation. Layout choices keep the
tensor engine free of bookkeeping work:

  - V is projected directly into [key, d] orientation (x tile stationary,
    wvT moving), so no transposes are needed before the PV matmul.
  - Scores are computed transposed ([key, q]) so softmax exp output feeds
    the PV matmul directly as the moving operand.
  - Softmax denominators are per-query sums over keys (= partitions). They
    are accumulated over key tiles on the vector engine, transposed through
    the DMA x-bar, and finished with a free-axis reduce - zero PE cycles.
  - Context normalization happens on [q, d]-transposed blocks with a
    per-partition reciprocal scalar, then transposes back for the output
    projection (context is tiny vs. probs, so this is cheap).
  - The output projection runs as dense PE-only phases (one per 1024-token
    half) with the stationary context tile reused across 4 matmuls.

Per-core kernel DRAM I/O (everything [partition=128, free]):
  xT   [2048, 2048] bf16   x[b].T             (feature k on partitions)
  wqT/wkT/wvT [2048, 512]  w[heads_slice].T   (k on partitions)
  woT  [512, 2048]  bf16   wo[:, slice].T     (local d on partitions)
  out  [2048, 2048] fp32   partial output for batch b
"""

import sys

if "/opt/trn_rl_repo" not in sys.path:
    sys.path.insert(0, "/opt/trn_rl_repo")

from contextlib import ExitStack

import ml_dtypes
import numpy as np

import concourse.bacc as bacc
import concourse.tile as tile
from concourse import mybir
from concourse.bass_utils import run_bass_kernel_spmd

BF16 = mybir.dt.bfloat16
F32 = mybir.dt.float32

B, S, DIM = 2, 2048, 2048
HEADS, HD = 16, 128
P = 128
N_CORES = 8
HGROUPS = 4  # head groups (second shard axis is batch)
HPC = HEADS // HGROUPS  # heads per core = 4
DL = HPC * HD  # local head dims per core = 512
SCALE = 1.0 / float(np.sqrt(HD))

NK = DIM // P  # 16 contraction tiles for the projections
NMC = S // 512  # 4 token chunks in the projection phase
NT = S // P  # 16 key tiles
QW = 1024  # query block width in the attention phase
NJB = S // QW  # 2 query blocks
NJT = QW // P  # 8 query tiles per block
NE = DIM // 512  # 4 output-dim chunks

_PROGRAM_CACHE = {}


def _emit(nc, tc, xT, wqT, wkT, wvT, woT, maskf, out):
    with_mask = maskf is not None
    EXP = mybir.ActivationFunctionType.Exp
    AXX = mybir.AxisListType.X
    ADD = mybir.AluOpType.add

    with ExitStack() as octx:
        planes = octx.enter_context(tc.tile_pool(name="planes", bufs=1))
        q_sb = [planes.tile([P, S], BF16, tag=f"q{h}", name=f"q{h}") for h in range(HPC)]
        k_sb = [planes.tile([P, S], BF16, tag=f"k{h}", name=f"k{h}") for h in range(HPC)]
        cx_sb = [planes.tile([P, S], BF16, tag=f"cx{h}", name=f"cx{h}") for h in range(HPC)]
        vv_sb = planes.tile([P, NT, DL], BF16, tag="vv", name="vv")
        wo_sb = planes.tile([P, HPC, DIM], BF16, tag="wo", name="wo")

        dmae = [nc.sync, nc.scalar, nc.gpsimd]

        # ---------------- Phase 1: QKV projections ----------------
        with ExitStack() as ctx:
            wpool = ctx.enter_context(tc.tile_pool(name="wqkv", bufs=1))
            w_sb = {
                name: wpool.tile([P, NK * DL], BF16, tag=f"w{name}", name=f"w{name}")
                for name in ("q", "k", "v")
            }
            xpool = ctx.enter_context(tc.tile_pool(name="xt", bufs=2 * NK))
            pq = ctx.enter_context(tc.tile_pool(name="ps_qkv", bufs=2, space="PSUM"))

            qi = 0
            for mc in range(NMC):
                xts = []
                for kt in range(NK):
                    t = xpool.tile([P, 512], BF16, tag="xt")
                    # kt-interleaved issue order across all three queues so
                    # delivery tracks the kt-major consumption order below
                    dmae[qi % 3].dma_start(
                        t[:], xT[kt * P : (kt + 1) * P, mc * 512 : (mc + 1) * 512]
                    )
                    qi += 1
                    xts.append(t)
                    if mc == 0:
                        dmae[qi % 3].dma_start(
                            w_sb["q"][:, kt * DL : (kt + 1) * DL],
                            wqT[kt * P : (kt + 1) * P, :],
                        )
                        qi += 1
                if mc == 0:
                    for name, srct in (("k", wkT), ("v", wvT)):
                        for kt in range(NK):
                            dmae[qi % 3].dma_start(
                                w_sb[name][:, kt * DL : (kt + 1) * DL],
                                srct[kt * P : (kt + 1) * P, :],
                            )
                            qi += 1
                # kt-major accumulation: 4 live PSUM banks per projection, so
                # each x tile is consumed right as it lands (no burst demand)
                for name, plane_list in (("q", q_sb), ("k", k_sb)):
                    pss = [
                        pq.tile([P, 512], F32, tag=f"ps{i}", name=f"ps{i}")
                        for i in range(4)
                    ]
                    for kt in range(NK):
                        for h in range(HPC):
                            nc.tensor.matmul(
                                pss[h][:],
                                w_sb[name][:, kt * DL + h * P : kt * DL + (h + 1) * P],
                                xts[kt][:],
                                start=(kt == 0),
                                stop=(kt == NK - 1),
                            )
                    for h in range(HPC):
                        nc.any.tensor_copy(
                            plane_list[h][:, mc * 512 : (mc + 1) * 512], pss[h][:]
                        )
                # v in [token(=key), d] orientation: x tile stationary
                pss = [
                    pq.tile([P, 512], F32, tag=f"ps{i}", name=f"ps{i}")
                    for i in range(4)
                ]
                for kt in range(NK):
                    for tt in range(4):
                        nc.tensor.matmul(
                            pss[tt][:],
                            xts[kt][:, tt * P : (tt + 1) * P],
                            w_sb["v"][:, kt * DL : (kt + 1) * DL],
                            start=(kt == 0),
                            stop=(kt == NK - 1),
                        )
                for tt in range(4):
                    nc.any.tensor_copy(vv_sb[:, mc * 4 + tt, :], pss[tt][:])

        # ------- Phase 2: attention (scoresT form), per query-block -------
        # scoresT(nt) = kT(nt)^T @ qT -> exp -> probsT [key, q] feeds PV
        # directly. Denominators: DVE accumulates probsT over key tiles,
        # x-bar transposes the sums, free-axis reduce + reciprocal gives
        # per-query scalars in column form. Context is normalized on [q, d]
        # blocks and transposed back. Out-projection runs as its own dense
        # PE phase after each query block.
        for jb in range(NJB):
            q0 = jb * QW
            with ExitStack() as ctx:
                ss_pool = ctx.enter_context(
                    tc.tile_pool(name="ss", bufs=3, space="PSUM")
                )
                cxp_pool = ctx.enter_context(
                    tc.tile_pool(name="cxp", bufs=1, space="PSUM")
                )
                pbt_pool = ctx.enter_context(tc.tile_pool(name="pbt", bufs=6))
                acc_pool = ctx.enter_context(tc.tile_pool(name="acc", bufs=2))
                tr_pool = ctx.enter_context(tc.tile_pool(name="tr", bufs=4))
                st_pool = ctx.enter_context(tc.tile_pool(name="st", bufs=4))
                if with_mask:
                    mpool = ctx.enter_context(tc.tile_pool(name="mask", bufs=3))

                if jb == 0:
                    # wo is first needed by the out-projection ~100us later;
                    # load it here so it never delays the phase-1 x/w DMAs
                    for h in range(HPC):
                        nc.gpsimd.dma_start(
                            wo_sb[:, h, :], woT[h * P : (h + 1) * P, :]
                        )

                for h in range(HPC):
                    # two bf16 accumulator chains for the softmax denominators
                    acc = [
                        acc_pool.tile([P, QW], BF16, tag=f"acc{i}", name=f"acc{i}")
                        for i in range(2)
                    ]
                    pbt01 = [None, None]
                    cxp = cxp_pool.tile([P, 2, 512], F32, tag="cxp", name="cxp")
                    for nt in range(NT):
                        ss = ss_pool.tile([P, QW], F32, tag="ss", name="ss")
                        kst = k_sb[h][:, nt * P : (nt + 1) * P]
                        nc.tensor.matmul(
                            ss[:, 0:512], kst, q_sb[h][:, q0 : q0 + 512],
                            start=True, stop=True,
                        )
                        nc.tensor.matmul(
                            ss[:, 512:1024], kst, q_sb[h][:, q0 + 512 : q0 + QW],
                            start=True, stop=True,
                        )
                        if with_mask:
                            mt = mpool.tile([P, QW], F32, tag="mt", name="mt")
                            nc.gpsimd.dma_start(
                                mt[:], maskf[nt * P : (nt + 1) * P, q0 : q0 + QW]
                            )
                            nc.vector.tensor_add(ss[:], ss[:], mt[:])
                        pbt = pbt_pool.tile([P, QW], BF16, tag="pbt", name="pbt")
                        nc.scalar.activation(pbt[:], ss[:], EXP, scale=SCALE)
                        a = acc[nt % 2]
                        if nt < 2:
                            pbt01[nt] = pbt  # chain seeded at nt=2/3 (saves 2 copies)
                        elif nt < 4:
                            nc.vector.tensor_add(a[:], pbt01[nt - 2][:], pbt[:])
                        else:
                            nc.vector.tensor_add(a[:], a[:], pbt[:])
                        vst = vv_sb[:, nt, h * P : (h + 1) * P]
                        nc.tensor.matmul(
                            cxp[:, 0, :], vst, pbt[:, 0:512],
                            start=(nt == 0), stop=(nt == NT - 1),
                        )
                        nc.tensor.matmul(
                            cxp[:, 1, :], vst, pbt[:, 512:1024],
                            start=(nt == 0), stop=(nt == NT - 1),
                        )
                    # tail: denominators + context normalization, in
                    # 512-wide halves so the first tokens' context lands
                    # quickly (shrinks the stall before the out-projection)
                    accb = acc_pool.tile([P, QW], BF16, tag="accb", name="accb")
                    nc.vector.tensor_add(accb[:], acc[0][:], acc[1][:])
                    for hv in range(2):
                        c0 = hv * 512
                        accT = tr_pool.tile([P, 4, P], BF16, tag="accT", name="accT")
                        nc.sync.dma_start(
                            accT[:], accb[:, c0 : c0 + 512], transpose=True
                        )
                        ctmp = acc_pool.tile([P, 512], BF16, tag="ctmp", name="ctmp")
                        nc.vector.tensor_copy(ctmp[:], cxp[:, hv, :])
                        cn = tr_pool.tile([P, 4, P], BF16, tag="cn", name="cn")
                        nc.sync.dma_start(cn[:], ctmp[:], transpose=True)
                        den = st_pool.tile([P, 4], F32, tag="den", name="den")
                        nc.vector.tensor_reduce(den[:], accT[:], axis=AXX, op=ADD)
                        rec = st_pool.tile([P, 4], F32, tag="rec", name="rec")
                        nc.vector.reciprocal(rec[:], den[:])
                        cnn = tr_pool.tile([P, 4, P], BF16, tag="cnn", name="cnn")
                        for j in range(4):
                            nc.vector.tensor_scalar_mul(
                                cnn[:, j, :], cn[:, j, :], rec[:, j : j + 1]
                            )
                        cx_dst = cx_sb[h][:, q0 + c0 : q0 + c0 + 512].rearrange(
                            "p (a b) -> p a b", a=4
                        )
                        nc.sync.dma_start(cx_dst, cnn[:], transpose=True)

            # ---- out projection for this query block (dense PE phase) ----
            with ExitStack() as ctx:
                po = ctx.enter_context(tc.tile_pool(name="po", bufs=2, space="PSUM"))
                obp = ctx.enter_context(tc.tile_pool(name="ob", bufs=6))
                for tt in range(NJT):
                    t0 = q0 + tt * P
                    pst = po.tile([P, NE, 512], F32, tag="pst", name="pst")
                    for h in range(HPC):
                        cst = cx_sb[h][:, t0 : t0 + P]
                        for ec in range(NE):
                            nc.tensor.matmul(
                                pst[:, ec, :],
                                cst,
                                wo_sb[:, h, ec * 512 : (ec + 1) * 512],
                                start=(h == 0),
                                stop=(h == HPC - 1),
                            )
                    for ec in range(NE):
                        # bf16 partial outputs (the host sums head-group
                        # partials in fp32): halves the output write traffic
                        ob = obp.tile([P, 512], BF16, tag="ob", name="ob")
                        # split PSUM->SBUF copies across vector+scalar so
                        # neither engine paces the PE here
                        if ec % 2:
                            nc.vector.tensor_copy(ob[:], pst[:, ec, :])
                        else:
                            nc.scalar.copy(ob[:], pst[:, ec, :])
                        dmae[(tt * NE + ec) % 3].dma_start(
                            out[t0 : t0 + P, ec * 512 : (ec + 1) * 512], ob[:]
                        )


def _build(with_mask: bool):
    nc = bacc.Bacc("TRN2")
    xT = nc.dram_tensor("xT", [DIM, S], BF16, kind="ExternalInput")
    wqT = nc.dram_tensor("wqT", [DIM, DL], BF16, kind="ExternalInput")
    wkT = nc.dram_tensor("wkT", [DIM, DL], BF16, kind="ExternalInput")
    wvT = nc.dram_tensor("wvT", [DIM, DL], BF16, kind="ExternalInput")
    woT = nc.dram_tensor("woT", [DL, DIM], BF16, kind="ExternalInput")
    maskf = (
        nc.dram_tensor("maskf", [S, S], F32, kind="ExternalInput")
        if with_mask
        else None
    )
    out = nc.dram_tensor("out", [S, DIM], BF16, kind="ExternalOutput")
    with tile.TileContext(nc) as tc:
        _emit(nc, tc, xT, wqT, wkT, wvT, woT, maskf, out)
    nc.finalize()
    return nc


def _get_program(with_mask: bool):
    if with_mask not in _PROGRAM_CACHE:
        _PROGRAM_CACHE[with_mask] = _build(with_mask)
    return _PROGRAM_CACHE[with_mask]


def _prep_in_maps(x, mask, wq, wk, wv, wo, with_mask):
    bf = ml_dtypes.bfloat16
    f32 = np.float32
    xTs = [np.ascontiguousarray(x[b].T.astype(bf)) for b in range(B)]
    if with_mask:
        maskf = np.ascontiguousarray(mask[0, 0].T.astype(f32) / SCALE)
    in_maps = []
    for c in range(N_CORES):
        b = c // HGROUPS
        g = c % HGROUPS
        sl = slice(g * DL, (g + 1) * DL)
        m = {
            "xT": xTs[b],
            "wqT": np.ascontiguousarray(wq[sl, :].T.astype(bf)),
            "wkT": np.ascontiguousarray(wk[sl, :].T.astype(bf)),
            "wvT": np.ascontiguousarray(wv[sl, :].T.astype(bf)),
            "woT": np.ascontiguousarray(wo[:, sl].T.astype(bf)),
        }
        if with_mask:
            m["maskf"] = maskf
        in_maps.append(m)
    return in_maps


def run_sharded(x, mask, wq, wk, wv, wo, trace=False, trace_kwargs=None):
    """Run the SPMD kernel; returns (full_output, BassKernelResults)."""
    with_mask = bool(np.any(np.asarray(mask)))
    nc = _get_program(with_mask)
    in_maps = _prep_in_maps(
        np.asarray(x), np.asarray(mask), np.asarray(wq), np.asarray(wk),
        np.asarray(wv), np.asarray(wo), with_mask,
    )
    kw = {}
    if trace:
        kw["trace"] = True
        if trace_kwargs:
            kw["trace_kwargs"] = trace_kwargs
    res = run_bass_kernel_spmd(nc, in_maps, list(range(N_CORES)), **kw)
    out = np.zeros((B, S, DIM), np.float32)
    for c in range(N_CORES):
        out[c // HGROUPS] += res.results[c]["out"].astype(np.float32)
    return out, res


def kernel(**inputs):
    out, _ = run_sharded(
        inputs["x"], inputs["mask"], inputs["wq"], inputs["wk"], inputs["wv"],
        inputs["wo"],
    )
    return out
